# revision 16
# baseline (speedup 1.0000x reference)
"""Angular prototypical loss on 8 TRN2 NeuronCores (Bass/Tile, SPMD).

kernel(**inputs): takes FULL inputs (embeddings [65536,256] f32, labels
[65536] i32, num_classes), shards the batch across the 8 cores, runs one
SPMD Bass kernel (AllReduce of per-class prototype sums on-chip), returns
the scalar mean loss.

v3 design (per core, 8192 rows = 64 tiles of 128):
- Phase A: f32->bf16 casting DMA load, one group-wide Square (ACT) +
  segmented DVE reduce for row norms, one-hot via DVE is_equal (2x mode),
  protoT[d, c] accumulated in 4 PSUM banks (moving dim 512).
- One 512KB bf16 AllReduce; prototype normalization on the transposed
  C-major layout (one xbar transpose each way).
- Phase B: cos for two row-tiles accumulates into a 4-bank PSUM pair,
  one Exp activation per pair ([128, 2048], fp16 out, bias -2.5 to keep
  fp16 sums in range), sumexp via fp16 segmented DVE reduce, m via
  per-tile indirect gather (issued all up-front) + TT dot.
"""
import math

import numpy as np

import concourse.bass as bass
import concourse.bacc as bacc
import concourse.mybir as mybir
import concourse.tile as tile
from concourse.bass_utils import run_bass_kernel_spmd

P = 128
D = 256
C = 1024
NCORES = 8
MARGIN = 0.2
INV_T = 10.0
EBIAS = -2.5          # exp(10*cos + EBIAS): keeps fp16 sums in range
COS_M = math.cos(MARGIN)
SIN_M = math.sin(MARGIN)
TH = math.cos(math.pi - MARGIN)

f32 = mybir.dt.float32
bf16 = mybir.dt.bfloat16
fp16 = mybir.dt.float16
i32 = mybir.dt.int32

AF = mybir.ActivationFunctionType
OP = mybir.AluOpType


def build(nt: int = 64, group: int = 8):
    BL = P * nt
    ng = nt // group
    assert nt % group == 0

    nc = bacc.Bacc("TRN2", target_bir_lowering=False, debug=False,
                   num_devices=NCORES)
    emb = nc.declare_dram_parameter("embeddings", [BL, D], f32, isOutput=False)
    lab = nc.declare_dram_parameter("labels", [BL], i32, isOutput=False)
    out = nc.declare_dram_parameter("out", [P, 1], f32, isOutput=True)

    emb_g = emb.ap().rearrange("(p q) d -> p q d", p=P)      # [128, nt, 256]
    lab_pn = lab.ap().rearrange("(p n) -> p n", p=P)         # [128, nt]

    with tile.TileContext(nc) as tc:
        with (
            tc.tile_pool(name="big", bufs=1) as big,
            tc.tile_pool(name="stg", bufs=2) as stg,
            tc.tile_pool(name="ohp", bufs=3) as ohp,
            tc.tile_pool(name="xpp", bufs=1) as xpp,
            tc.tile_pool(name="scr", bufs=2) as scr,
            tc.tile_pool(name="dram", bufs=1, space="DRAM") as dram,
        ):
            s_loc = dram.tile([P, 2, C], bf16, tag="s_loc")
            s_glob = dram.tile([P, 2, C], bf16, tag="s_glob",
                               addr_space="Shared")
            shat_dram = dram.tile([C, D], bf16, tag="shat_dram")

            # ---- persistent SBUF ----
            ehat = big.tile([P, nt, D], bf16, tag="ehat")
            eT = big.tile([P, nt, 2, P], bf16, tag="eT")
            G_all = big.tile([P, nt, D], bf16, tag="G_all")
            sT = big.tile([P, 2, C], bf16, tag="sT")
            lab_i = big.tile([P, nt], i32, tag="lab_i")
            lab_f = big.tile([P, nt], f32, tag="lab_f")
            normsq = big.tile([P, nt], f32, tag="normsq")
            invn = big.tile([P, nt], f32, tag="invn")
            m_all = big.tile([P, nt], f32, tag="m_all")
            sumexp = big.tile([P, nt], fp16, tag="sumexp")
            iota16 = big.tile([P, C], fp16, tag="iota16")

            ebias_t = big.tile([P, 1], f32, tag="ebias_t")
            nc.vector.memset(ebias_t[:], EBIAS)
            nc.gpsimd.iota(iota16[:], pattern=[[1, C]], base=0,
                           channel_multiplier=0,
                           allow_small_or_imprecise_dtypes=True)
            nc.sync.dma_start(out=lab_i[:], in_=lab_pn)
            nc.vector.tensor_copy(lab_f[:], lab_i[:])

            # ================= Phase A =================
            with tc.tile_pool(name="psA", bufs=1, space="PSUM") as psA:
                proto_ps = [[psA.tile([P, C // 2], f32, tag=f"proto{ch}{h}",
                                      name=f"proto_ps{ch}{h}")
                             for h in range(2)] for ch in range(2)]
                for g in range(ng):
                    gsl = slice(g * group, (g + 1) * group)
                    ebf = stg.tile([P, group, D], bf16, tag="ebf")
                    # f32 -> bf16 casting DMA (gpsimd SWDGE casts)
                    nc.gpsimd.dma_start(out=ebf[:], in_=emb_g[:, gsl, :])
                    # row norms: one wide Square + segmented reduce
                    sq_g = scr.tile([P, group, D], bf16, tag="sq_g")
                    nc.scalar.activation(
                        sq_g[:].rearrange("p g d -> p (g d)"),
                        ebf[:].rearrange("p g d -> p (g d)"), AF.Square)
                    nc.vector.reduce_sum(normsq[:, gsl], sq_g[:],
                                         axis=mybir.AxisListType.X)
                    tmp8 = scr.tile([P, group], f32, tag="tmp8")
                    nc.vector.reciprocal(tmp8[:], normsq[:, gsl])
                    nc.scalar.sqrt(invn[:, gsl], tmp8[:])
                    for t in range(group):
                        n = g * group + t
                        e_n = ehat[:, n, :]
                        nc.vector.tensor_scalar(
                            e_n, ebf[:, t, :], invn[:, n:n + 1], None, OP.mult)
                        oh = ohp.tile([P, C], bf16, tag="oh")
                        nc.vector.tensor_scalar(
                            oh[:], iota16[:], lab_f[:, n:n + 1], None,
                            OP.is_equal)
                        for ch in range(2):
                            for h in range(2):
                                nc.tensor.matmul(
                                    out=proto_ps[ch][h][:],
                                    lhsT=e_n[:, ch * P:(ch + 1) * P],
                                    rhs=oh[:, h * 512:(h + 1) * 512],
                                    start=(n == 0), stop=(n == nt - 1))
                    nc.sync.dma_start_transpose(
                        out=eT[:, gsl, :, :],
                        in_=ehat[:, gsl, :].rearrange("p g d -> p (g d)"))

                # ---- PSUM -> SBUF bf16: s_sb[p, ch, c] = proto[c, 128ch+p]
                s_sb = big.tile([P, 2, C], bf16, tag="s_sb")
                for ch in range(2):
                    for h in range(2):
                        nc.vector.tensor_copy(
                            s_sb[:, ch, h * 512:(h + 1) * 512],
                            proto_ps[ch][h][:])

            # ---- single AllReduce ----
            nc.sync.dma_start(out=s_loc[:], in_=s_sb[:])
            nc.gpsimd.collective_compute(
                "AllReduce", OP.add,
                replica_groups=[list(range(NCORES))],
                ins=[s_loc[:].opt()], outs=[s_glob[:].opt()])
            s2 = xpp.tile([P, 2, C], bf16, tag="s2")
            nc.sync.dma_start(out=s2[:], in_=s_glob[:])

            # ---- normalize prototypes in C-major layout ----
            # sC[c, ch, cc, p] = proto[128cc + c, 128ch + p]
            sC = xpp.tile([P, 2, 8, P], bf16, tag="sC")
            nc.sync.dma_start_transpose(
                out=sC[:].rearrange("c ch cc p -> c (ch cc) p"),
                in_=s2[:].rearrange("p ch c -> p (ch c)"))
            sqC = xpp.tile([P, 2, 8, P], bf16, tag="sqC")
            nc.vector.tensor_tensor(sqC[:], sC[:], sC[:], op=OP.mult)
            pnsq = xpp.tile([P, 8], f32, tag="pnsq")
            nc.vector.reduce_sum(
                pnsq[:], sqC[:].rearrange("c ch cc p -> c cc ch p"),
                axis=mybir.AxisListType.XY)
            ptmp = xpp.tile([P, 8], f32, tag="ptmp")
            pinv = xpp.tile([P, 8], f32, tag="pinv")
            nc.vector.reciprocal(ptmp[:], pnsq[:])
            nc.scalar.sqrt(pinv[:], ptmp[:])
            shatC = xpp.tile([P, 2, 8, P], bf16, tag="shatC")
            for cc in range(8):
                nc.vector.tensor_scalar(
                    shatC[:, :, cc, :], sC[:, :, cc, :],
                    pinv[:, cc:cc + 1], None, OP.mult)
            # gather table rows (class-major, natural d order)
            for ch in range(2):
                nc.sync.dma_start(
                    out=shat_dram[:].rearrange(
                        "(cc c) (ch p) -> ch c cc p", c=P, ch=2)[ch],
                    in_=shatC[:, ch, :, :])
            # back to d-major for the Phase B rhs
            for ch in range(2):
                nc.sync.dma_start_transpose(
                    out=sT[:, ch, :].rearrange("p (cc c) -> p cc c", c=P),
                    in_=shatC[:, ch, :, :].rearrange("c cc p -> c (cc p)"))

            # ---- all target-prototype gathers up-front ----
            for n in range(nt):
                nc.gpsimd.indirect_dma_start(
                    out=G_all[:, n, :], out_offset=None,
                    in_=shat_dram[:],
                    in_offset=bass.IndirectOffsetOnAxis(
                        ap=lab_i[:, n:n + 1], axis=0))

            # ================= Phase B =================
            with tc.tile_pool(name="psB", bufs=2, space="PSUM") as psB:
                for pr in range(nt // 2):
                    pp = psB.tile([P, 2, C], f32, tag="pp")
                    for t in range(2):
                        n = pr * 2 + t
                        for ch in range(2):
                            for hh in range(2):
                                nc.tensor.matmul(
                                    out=pp[:, t, hh * 512:(hh + 1) * 512],
                                    lhsT=eT[:, n, ch, :],
                                    rhs=sT[:, ch, hh * 512:(hh + 1) * 512],
                                    start=(ch == 0), stop=(ch == 1))
                    exps = scr.tile([P, 2, C], fp16, tag="exps")
                    nc.scalar.activation(
                        exps[:].rearrange("p t c -> p (t c)"),
                        pp[:].rearrange("p t c -> p (t c)"),
                        AF.Exp, bias=ebias_t[:], scale=INV_T)
                    with nc.allow_low_precision(reason="fp16 sumexp"):
                        nc.vector.reduce_sum(
                            sumexp[:, pr * 2:pr * 2 + 2], exps[:],
                            axis=mybir.AxisListType.X)
                    for t in range(2):
                        n = pr * 2 + t
                        mdf = scr.tile([P, D], bf16, tag="mdf")
                        nc.vector.tensor_tensor(
                            mdf[:], ehat[:, n, :], G_all[:, n, :],
                            op=OP.mult)
                        nc.vector.reduce_sum(m_all[:, n:n + 1], mdf[:],
                                             axis=mybir.AxisListType.X)

            # ================= epilogue (batched [P, nt]) ========
            b1 = big.tile([P, nt], f32, tag="b1")
            b2 = big.tile([P, nt], f32, tag="b2")
            b3 = big.tile([P, nt], f32, tag="b3")
            b4 = big.tile([P, nt], f32, tag="b4")
            mask = big.tile([P, nt], mybir.dt.uint8, tag="mask")
            phi_f = big.tile([P, nt], f32, tag="phi_f")

            nc.vector.tensor_tensor(b1[:], m_all[:], m_all[:], op=OP.mult)
            nc.vector.tensor_scalar(b1[:], b1[:], -1.0, 1.0, OP.mult, OP.add)
            nc.vector.tensor_scalar_max(b1[:], b1[:], 0.0)
            nc.scalar.sqrt(b2[:], b1[:])                        # sin
            nc.vector.tensor_scalar_mul(b3[:], m_all[:], COS_M)
            nc.vector.tensor_scalar(b2[:], b2[:], -SIN_M, None, OP.mult)
            nc.vector.tensor_add(b3[:], b3[:], b2[:])           # phi
            nc.vector.tensor_scalar(mask[:], m_all[:], TH, None, OP.is_gt)
            nc.vector.tensor_scalar(b4[:], m_all[:], -MARGIN, None, OP.add)
            nc.vector.select(phi_f[:], mask[:], b3[:], b4[:])
            nc.scalar.activation(b1[:], m_all[:], AF.Exp, bias=ebias_t[:],
                                 scale=INV_T)
            nc.scalar.activation(b2[:], phi_f[:], AF.Exp, bias=ebias_t[:],
                                 scale=INV_T)
            nc.vector.tensor_sub(b1[:], sumexp[:], b1[:])
            nc.vector.tensor_add(b1[:], b1[:], b2[:])           # Z * e^EBIAS
            nc.scalar.activation(b2[:], b1[:], AF.Ln, scale=1.0)
            # nll = (lnZ' - EBIAS) - 10*phi
            nc.vector.tensor_scalar_mul(b3[:], phi_f[:], INV_T)
            nc.vector.tensor_sub(b2[:], b2[:], b3[:])
            nc.vector.tensor_scalar(b2[:], b2[:], -EBIAS, None, OP.add)
            part = big.tile([P, 1], f32, tag="part")
            nc.vector.reduce_sum(part[:], b2[:], axis=mybir.AxisListType.X)
            nc.sync.dma_start(out=out[:], in_=part[:])

    nc.compile()
    return nc


_NC_CACHE = {}


def kernel(embeddings, labels, num_classes=None, **_ignored):
    embeddings = np.ascontiguousarray(embeddings, dtype=np.float32)
    labels = np.ascontiguousarray(labels, dtype=np.int32)
    B = embeddings.shape[0]
    BL = B // NCORES

    if "nc" not in _NC_CACHE:
        _NC_CACHE["nc"] = build()
    nc = _NC_CACHE["nc"]

    in_maps = [{"embeddings": embeddings[i * BL:(i + 1) * BL],
                "labels": labels[i * BL:(i + 1) * BL]}
               for i in range(NCORES)]
    res = run_bass_kernel_spmd(nc, in_maps, list(range(NCORES)))
    total = 0.0
    for i in range(NCORES):
        total += res.results[i]["out"].astype(np.float64).sum()
    return np.float32(total / B)


# revision 19
# speedup vs baseline: 1.0108x; 1.0108x over previous
"""Angular prototypical loss on 8 TRN2 NeuronCores (Bass/Tile, SPMD).

kernel(**inputs): takes FULL inputs (embeddings [65536,256] f32, labels
[65536] i32, num_classes), shards the batch across the 8 cores, runs one
SPMD Bass kernel (AllReduce of per-class prototype sums on-chip), returns
the scalar mean loss.

v4 design (per core, 8192 rows = 64 tiles of 128):
- Embeddings stay UNNORMALIZED: 1/||e|| folds into the one-hot (Phase A),
  the Exp scale (Phase B) and the m epilogue, killing a DVE pass.
- Phase A: all casting DMAs up-front into a persistent bf16 copy; row
  norms via one group-wide Square (ACT) + segmented DVE reduce; one-hot
  as a dual-op tensor_scalar (is_equal then *invn, 2x mode); protoT
  accumulated in 4 PSUM banks.
- The AllReduce payload is CLASS-MAJOR [C, D] (transpose moved before
  the AR), so the 64 per-tile target-row gathers read the AR output
  directly and start the moment it lands; rows are raw sums, so m is
  rescaled by rsqrt(sum G^2) in the epilogue.
- Phase B: 4 matmuls per tile into a 2-bank PSUM tile (4-deep pool),
  per-tile Exp with per-partition scale invn*10; sumexp fused in the
  ACT accumulator for most tiles and via a DVE tensor_scalar
  accumulate for a subset (engine balancing); m and sum(G^2) via
  one fused STT-accumulate each.
"""
import math

import numpy as np

import concourse.bass as bass
import concourse.bacc as bacc
import concourse.mybir as mybir
import concourse.tile as tile
from concourse.bass_utils import run_bass_kernel_spmd

P = 128
D = 256
C = 1024
NCORES = 8
MARGIN = 0.2
INV_T = 10.0
COS_M = math.cos(MARGIN)
SIN_M = math.sin(MARGIN)
TH = math.cos(math.pi - MARGIN)

f32 = mybir.dt.float32
bf16 = mybir.dt.bfloat16
fp16 = mybir.dt.float16
i32 = mybir.dt.int32

AF = mybir.ActivationFunctionType
OP = mybir.AluOpType

# tiles whose sumexp accumulates on DVE instead of the ACT accumulator
DVE_SUMEXP = 3  # out of every 8


def build(nt: int = 64, group: int = 8):
    BL = P * nt
    ng = nt // group
    assert nt % group == 0

    nc = bacc.Bacc("TRN2", target_bir_lowering=False, debug=False,
                   num_devices=NCORES)
    emb = nc.declare_dram_parameter("embeddings", [BL, D], f32, isOutput=False)
    lab = nc.declare_dram_parameter("labels", [BL], i32, isOutput=False)
    out = nc.declare_dram_parameter("out", [P, 1], f32, isOutput=True)

    emb_g = emb.ap().rearrange("(p q) d -> p q d", p=P)      # [128, nt, 256]
    lab_pn = lab.ap().rearrange("(p n) -> p n", p=P)         # [128, nt]

    with tile.TileContext(nc) as tc:
        with (
            tc.tile_pool(name="big", bufs=1) as big,
            tc.tile_pool(name="ohp", bufs=3) as ohp,
            tc.tile_pool(name="xpp", bufs=1) as xpp,
            tc.tile_pool(name="scr", bufs=2) as scr,
            tc.tile_pool(name="dram", bufs=1, space="DRAM") as dram,
        ):
            # class-major AllReduce staging: rows are classes
            s_loc = dram.tile([C, D], bf16, tag="s_loc")
            s_glob = dram.tile([C, D], bf16, tag="s_glob",
                               addr_space="Shared")

            # ---- persistent SBUF ----
            e_all = big.tile([P, nt, D], bf16, tag="e_all")   # raw bf16
            eT = big.tile([P, nt, 2, P], bf16, tag="eT")
            G_all = big.tile([P, nt, D], bf16, tag="G_all")   # raw proto rows
            sT = big.tile([P, 2, C], bf16, tag="sT")
            lab_i = big.tile([P, nt], i32, tag="lab_i")
            lab_f = big.tile([P, nt], f32, tag="lab_f")
            normsq = big.tile([P, nt], f32, tag="normsq")
            invn = big.tile([P, nt], f32, tag="invn")
            invn10 = big.tile([P, nt], f32, tag="invn10")
            m_raw = big.tile([P, nt], f32, tag="m_raw")
            gsq = big.tile([P, nt], f32, tag="gsq")
            sumexp = big.tile([P, nt], f32, tag="sumexp")
            iota16 = big.tile([P, C], fp16, tag="iota16")
            ones16 = big.tile([P, C], fp16, tag="ones16")
            nc.vector.memset(ones16[:], 1.0)

            nc.gpsimd.iota(iota16[:], pattern=[[1, C]], base=0,
                           channel_multiplier=0,
                           allow_small_or_imprecise_dtypes=True)
            nc.sync.dma_start(out=lab_i[:], in_=lab_pn)
            nc.vector.tensor_copy(lab_f[:], lab_i[:])

            # all embedding loads up-front (f32 -> bf16 casting DMAs)
            for g in range(ng):
                gsl = slice(g * group, (g + 1) * group)
                nc.gpsimd.dma_start(out=e_all[:, gsl, :],
                                    in_=emb_g[:, gsl, :])

            def stats(g):
                gsl = slice(g * group, (g + 1) * group)
                sq_g = scr.tile([P, group, D], bf16, tag="sq_g")
                nc.scalar.activation(
                    sq_g[:].rearrange("p g d -> p (g d)"),
                    e_all[:, gsl, :].rearrange("p g d -> p (g d)"),
                    AF.Square)
                nc.vector.reduce_sum(normsq[:, gsl], sq_g[:],
                                     axis=mybir.AxisListType.X)
                tmp8 = scr.tile([P, group], f32, tag="tmp8")
                nc.vector.reciprocal(tmp8[:], normsq[:, gsl])
                nc.scalar.sqrt(invn[:, gsl], tmp8[:])
                nc.vector.tensor_scalar_mul(invn10[:, gsl], invn[:, gsl],
                                            INV_T)

            # ================= Phase A =================
            with tc.tile_pool(name="psA", bufs=1, space="PSUM") as psA:
                proto_ps = [[psA.tile([P, C // 2], f32, tag=f"proto{ch}{h}",
                                      name=f"proto_ps{ch}{h}")
                             for h in range(2)] for ch in range(2)]
                stats(0)
                for g in range(ng):
                    if g + 1 < ng:
                        stats(g + 1)
                    for t in range(group):
                        n = g * group + t
                        oh = ohp.tile([P, C], bf16, tag="oh")
                        nc.vector.tensor_scalar(
                            oh[:], iota16[:], lab_f[:, n:n + 1],
                            invn[:, n:n + 1], OP.is_equal, OP.mult)
                        for ch in range(2):
                            for h in range(2):
                                nc.tensor.matmul(
                                    out=proto_ps[ch][h][:],
                                    lhsT=e_all[:, n, ch * P:(ch + 1) * P],
                                    rhs=oh[:, h * 512:(h + 1) * 512],
                                    start=(n == 0), stop=(n == nt - 1))
                    gsl = slice(g * group, (g + 1) * group)
                    nc.sync.dma_start_transpose(
                        out=eT[:, gsl, :, :],
                        in_=e_all[:, gsl, :].rearrange("p g d -> p (g d)"))

                # PSUM -> SBUF bf16: s_sb[p, ch, c] = proto[c, 128ch+p]
                s_sb = big.tile([P, 2, C], bf16, tag="s_sb")
                for ch in range(2):
                    for h in range(2):
                        nc.scalar.copy(
                            s_sb[:, ch, h * 512:(h + 1) * 512],
                            proto_ps[ch][h][:])

            # ---- transpose to class-major BEFORE the AR ----
            # sCm[c, ch, cc, p] = proto[128cc + c, 128ch + p]
            sCm = xpp.tile([P, 2, 8, P], bf16, tag="sCm")
            nc.sync.dma_start_transpose(
                out=sCm[:].rearrange("c ch cc p -> c (ch cc) p"),
                in_=s_sb[:].rearrange("p ch c -> p (ch c)"))
            for ch in range(2):
                nc.sync.dma_start(
                    out=s_loc[:].rearrange(
                        "(cc c) (ch p) -> ch c cc p", c=P, ch=2)[ch],
                    in_=sCm[:, ch, :, :])
            nc.gpsimd.collective_compute(
                "AllReduce", OP.add,
                replica_groups=[list(range(NCORES))],
                ins=[s_loc[:].opt()], outs=[s_glob[:].opt()])

            # ---- raw-row gathers straight from the AR output ----
            for n in range(nt):
                nc.gpsimd.indirect_dma_start(
                    out=G_all[:, n, :], out_offset=None,
                    in_=s_glob[:],
                    in_offset=bass.IndirectOffsetOnAxis(
                        ap=lab_i[:, n:n + 1], axis=0))

            # ---- normalize prototypes (class-major, no fwd transpose) ----
            # sC2[c, dh, cc, ds] = protosum[128cc + c, 128dh + ds]
            sC2 = xpp.tile([P, 2, 8, P], bf16, tag="sC2")
            for dh in range(2):
                nc.sync.dma_start(
                    out=sC2[:, dh, :, :],
                    in_=s_glob[:].rearrange(
                        "(cc c) (dh ds) -> dh c cc ds", c=P, dh=2)[dh])
            pnsq = xpp.tile([P, 8], f32, tag="pnsq")
            for cc in range(8):
                sqd = scr.tile([P, 2, P], bf16, tag="sqd")
                nc.vector.scalar_tensor_tensor(
                    out=sqd[:], in0=sC2[:, :, cc, :], scalar=1.0,
                    in1=sC2[:, :, cc, :], op0=OP.mult, op1=OP.mult,
                    accum_out=pnsq[:, cc:cc + 1])
            ptmp = xpp.tile([P, 8], f32, tag="ptmp")
            pinv = xpp.tile([P, 8], f32, tag="pinv")
            nc.vector.reciprocal(ptmp[:], pnsq[:])
            nc.scalar.sqrt(pinv[:], ptmp[:])
            shatC = xpp.tile([P, 2, 8, P], bf16, tag="shatC")
            for cc in range(8):
                nc.vector.tensor_scalar(
                    shatC[:, :, cc, :], sC2[:, :, cc, :],
                    pinv[:, cc:cc + 1], None, OP.mult)
            # one transpose back to d-major: blocks (dh, cc) -> sT
            nc.sync.dma_start_transpose(
                out=sT[:].rearrange("p ch (cc c) -> p (ch cc) c", c=P),
                in_=shatC[:].rearrange("c dh cc ds -> c (dh cc ds)"))

            # ================= Phase B =================
            with tc.tile_pool(name="psB", bufs=4, space="PSUM") as psB:
                for n in range(nt):
                    pp = psB.tile([P, C], f32, tag="pp")
                    for ch in range(2):
                        for hh in range(2):
                            nc.tensor.matmul(
                                out=pp[:, hh * 512:(hh + 1) * 512],
                                lhsT=eT[:, n, ch, :],
                                rhs=sT[:, ch, hh * 512:(hh + 1) * 512],
                                start=(ch == 0), stop=(ch == 1))
                    exps = scr.tile([P, C], fp16, tag="exps")
                    if n % 8 < DVE_SUMEXP:
                        nc.scalar.activation(
                            exps[:], pp[:], AF.Exp,
                            scale=invn10[:, n:n + 1])
                        dum = scr.tile([P, C], fp16, tag="dum")
                        nc.vector.scalar_tensor_tensor(
                            out=dum[:], in0=exps[:], scalar=1.0,
                            in1=ones16[:], op0=OP.mult, op1=OP.mult,
                            accum_out=sumexp[:, n:n + 1])
                    else:
                        nc.scalar.activation(
                            exps[:], pp[:], AF.Exp,
                            scale=invn10[:, n:n + 1],
                            accum_out=sumexp[:, n:n + 1])
                    mdf = scr.tile([P, D], bf16, tag="mdf")
                    nc.vector.scalar_tensor_tensor(
                        out=mdf[:], in0=e_all[:, n, :], scalar=1.0,
                        in1=G_all[:, n, :], op0=OP.mult, op1=OP.mult,
                        accum_out=m_raw[:, n:n + 1])
                    gdf = scr.tile([P, D], bf16, tag="gdf")
                    nc.vector.scalar_tensor_tensor(
                        out=gdf[:], in0=G_all[:, n, :], scalar=1.0,
                        in1=G_all[:, n, :], op0=OP.mult, op1=OP.mult,
                        accum_out=gsq[:, n:n + 1])

            # ================= epilogue (batched [P, nt]) ========
            b1 = big.tile([P, nt], f32, tag="b1")
            b2 = big.tile([P, nt], f32, tag="b2")
            b3 = big.tile([P, nt], f32, tag="b3")
            b4 = big.tile([P, nt], f32, tag="b4")
            mask = big.tile([P, nt], mybir.dt.uint8, tag="mask")
            phi_f = big.tile([P, nt], f32, tag="phi_f")
            m_all = big.tile([P, nt], f32, tag="m_all")

            # m = m_raw * invn * rsqrt(gsq)
            nc.vector.reciprocal(b1[:], gsq[:])
            nc.scalar.sqrt(b2[:], b1[:])
            nc.vector.tensor_tensor(b3[:], m_raw[:], invn[:], op=OP.mult)
            nc.vector.tensor_tensor(m_all[:], b3[:], b2[:], op=OP.mult)

            nc.vector.tensor_tensor(b1[:], m_all[:], m_all[:], op=OP.mult)
            nc.vector.tensor_scalar(b1[:], b1[:], -1.0, 1.0, OP.mult, OP.add)
            nc.vector.tensor_scalar_max(b1[:], b1[:], 0.0)
            nc.scalar.sqrt(b2[:], b1[:])                        # sin
            nc.vector.tensor_scalar_mul(b3[:], m_all[:], COS_M)
            nc.vector.tensor_scalar(b2[:], b2[:], -SIN_M, None, OP.mult)
            nc.vector.tensor_add(b3[:], b3[:], b2[:])           # phi
            nc.vector.tensor_scalar(mask[:], m_all[:], TH, None, OP.is_gt)
            nc.vector.tensor_scalar(b4[:], m_all[:], -MARGIN, None, OP.add)
            nc.vector.select(phi_f[:], mask[:], b3[:], b4[:])
            nc.scalar.activation(b1[:], m_all[:], AF.Exp, scale=INV_T)
            nc.scalar.activation(b2[:], phi_f[:], AF.Exp, scale=INV_T)
            nc.vector.tensor_sub(b1[:], sumexp[:], b1[:])
            nc.vector.tensor_add(b1[:], b1[:], b2[:])           # Z
            nc.scalar.activation(b2[:], b1[:], AF.Ln, scale=1.0)
            nc.vector.tensor_scalar_mul(b3[:], phi_f[:], INV_T)
            nc.vector.tensor_sub(b2[:], b2[:], b3[:])           # nll
            part = big.tile([P, 1], f32, tag="part")
            nc.vector.reduce_sum(part[:], b2[:], axis=mybir.AxisListType.X)
            nc.sync.dma_start(out=out[:], in_=part[:])

    nc.compile()
    return nc


_NC_CACHE = {}


def kernel(embeddings, labels, num_classes=None, **_ignored):
    embeddings = np.ascontiguousarray(embeddings, dtype=np.float32)
    labels = np.ascontiguousarray(labels, dtype=np.int32)
    B = embeddings.shape[0]
    BL = B // NCORES

    if "nc" not in _NC_CACHE:
        _NC_CACHE["nc"] = build()
    nc = _NC_CACHE["nc"]

    in_maps = [{"embeddings": embeddings[i * BL:(i + 1) * BL],
                "labels": labels[i * BL:(i + 1) * BL]}
               for i in range(NCORES)]
    res = run_bass_kernel_spmd(nc, in_maps, list(range(NCORES)))
    total = 0.0
    for i in range(NCORES):
        total += res.results[i]["out"].astype(np.float64).sum()
    return np.float32(total / B)


# revision 20
# speedup vs baseline: 1.2903x; 1.2766x over previous
"""Angular prototypical loss on 8 TRN2 NeuronCores (Bass/Tile, SPMD).

kernel(**inputs): takes FULL inputs (embeddings [65536,256] f32, labels
[65536] i32, num_classes), shards the batch across the 8 cores, runs one
SPMD Bass kernel (AllReduce of per-class prototype sums on-chip), returns
the scalar mean loss. See build() for the per-core algorithm.
"""
import numpy as np
from concourse.bass_utils import run_bass_kernel_spmd

import math

import concourse.bass as bass
import concourse.mybir as mybir
import concourse.tile as tile
import concourse.bacc as bacc

P = 128
D = 256
C = 1024
NCORES = 8
MARGIN = 0.2
INV_T = 10.0
COS_M = math.cos(MARGIN)
SIN_M = math.sin(MARGIN)
TH = math.cos(math.pi - MARGIN)

f32 = mybir.dt.float32
bf16 = mybir.dt.bfloat16
fp16 = mybir.dt.float16
i32 = mybir.dt.int32
fp8 = mybir.dt.float8e4

AF = mybir.ActivationFunctionType
OP = mybir.AluOpType


def build(nt: int = 64, group: int = 8):
    """nt: row-tiles per core (rows/core = 128*nt). group: tiles per DMA group."""
    BL = P * nt
    ng = nt // group
    assert nt % group == 0

    nc = bacc.Bacc("TRN2", target_bir_lowering=False, debug=False,
                   num_devices=NCORES)
    emb = nc.declare_dram_parameter("embeddings", [BL, D], f32, isOutput=False)
    lab = nc.declare_dram_parameter("labels", [BL], i32, isOutput=False)
    out = nc.declare_dram_parameter("out", [P, 1], f32, isOutput=True)

    emb_g = emb.ap().rearrange("(p q) d -> p q d", p=P)      # [128, nt, 256]
    lab_pn = lab.ap().rearrange("(p n) -> p n", p=P)         # [128, nt]

    with tile.TileContext(nc) as tc:
        with (
            tc.tile_pool(name="big", bufs=1) as big,
            tc.tile_pool(name="stage", bufs=2) as stage,
            tc.tile_pool(name="ohp", bufs=4) as ohp,
            tc.tile_pool(name="gat", bufs=2) as gat,
            tc.tile_pool(name="scr", bufs=2) as scr,
            tc.tile_pool(name="dram", bufs=1, space="DRAM") as dram,
        ):
            s_local = dram.tile([C, D], bf16, tag="s_local")
            s_global = dram.tile([C, D], bf16, tag="s_global",
                                 addr_space="Shared")
            shat_dram = dram.tile([C, D], bf16, tag="shat_dram")
            s_local_v = s_local.rearrange("(j p) d -> p j d", p=P)
            s_global_v = s_global.rearrange("(j p) d -> p j d", p=P)
            shat_v = shat_dram.rearrange("(j p) d -> p j d", p=P)
            # ---- persistent SBUF ----
            e_bf = big.tile([P, nt * D], bf16, tag="e_bf")
            eT = big.tile([P, nt, 2, P], bf16, tag="eT")
            sT = big.tile([P, 2, C], bf16, tag="sT")
            lab_i = big.tile([P, nt], i32, tag="lab_i")
            lab_f = big.tile([P, nt], f32, tag="lab_f")
            normsq = big.tile([P, nt], f32, tag="normsq")
            invn = big.tile([P, nt], f32, tag="invn")
            m_all = big.tile([P, nt], f32, tag="m_all")
            sumexp = big.tile([P, nt], f32, tag="sumexp")
            iota16 = big.tile([P, C], fp16, tag="iota16")

            nc.gpsimd.iota(iota16[:], pattern=[[1, C]], base=0,
                           channel_multiplier=0,
                           allow_small_or_imprecise_dtypes=True)
            nc.sync.dma_start(out=lab_i[:], in_=lab_pn)
            nc.vector.tensor_copy(lab_f[:], lab_i[:])

            # ================= Phase A =================
            # Software-pipelined groups: stats (DMA + rownorm) for group g
            # are issued before the consume stage (normalize+onehot+matmul)
            # of group g-1, so the tiny reciprocal isn't queued behind bulk
            # DVE work and engines overlap across stages.
            with tc.tile_pool(name="psA", bufs=1, space="PSUM") as psA:
                proto_ps = [psA.tile([P, D], f32, tag=f"proto{j}",
                                     name=f"proto_ps{j}")
                            for j in range(8)]
                for g in range(ng):
                    raw = stage.tile([P, group, D], f32, tag="raw")
                    nc.sync.dma_start(out=raw[:],
                                      in_=emb_g[:, g * group:(g + 1) * group, :])
                    for t in range(group):
                        n = g * group + t
                        sq = scr.tile([P, D], f32, tag="sq")
                        nc.scalar.activation(
                            sq[:], raw[:, t, :], AF.Square,
                            accum_out=normsq[:, n:n + 1])
                    tmp8 = scr.tile([P, group], f32, tag="tmp8")
                    gsl = slice(g * group, (g + 1) * group)
                    nc.vector.reciprocal(tmp8[:], normsq[:, gsl])
                    nc.scalar.sqrt(invn[:, gsl], tmp8[:])
                    for t in range(group):
                        n = g * group + t
                        e_n = e_bf[:, n * D:(n + 1) * D]
                        nc.vector.tensor_scalar(
                            e_n, raw[:, t, :], invn[:, n:n + 1], None, OP.mult)
                        oh = ohp.tile([P, C], bf16, tag="oh")
                        nc.vector.tensor_scalar(
                            oh[:], iota16[:], lab_f[:, n:n + 1], None,
                            OP.is_equal)
                        for j in range(8):
                            nc.tensor.matmul(
                                out=proto_ps[j][:],
                                lhsT=oh[:, j * P:(j + 1) * P],
                                rhs=e_n,
                                start=(n == 0), stop=(n == nt - 1))
                    # one batched xbar transpose for the whole group
                    nc.sync.dma_start_transpose(
                        out=eT[:, g * group:(g + 1) * group, :, :],
                        in_=e_bf[:, g * group * D:(g + 1) * group * D])

                # ---- proto epilogue: PSUM -> SBUF (bf16 for cheap AR) ----
                s_sb = big.tile([P, 8, D], bf16, tag="s_sb")
                for j in range(8):
                    nc.scalar.copy(s_sb[:, j, :], proto_ps[j][:])

            # ---- DRAM -> AllReduce -> back ----
            nc.sync.dma_start(out=s_local_v, in_=s_sb[:])
            nc.gpsimd.collective_compute(
                "AllReduce", OP.add,
                replica_groups=[list(range(NCORES))],
                ins=[s_local[:].opt()], outs=[s_global[:].opt()])
            s_sb2 = big.tile([P, 8, D], bf16, tag="s_sb2")
            nc.sync.dma_start(out=s_sb2[:], in_=s_global_v)

            # ---- normalize prototypes ----
            pnsq = big.tile([P, 8], f32, tag="pnsq")
            pinv = big.tile([P, 8], f32, tag="pinv")
            ptmp = big.tile([P, 8], f32, tag="ptmp")
            shat_sb = big.tile([P, 8, D], bf16, tag="shat_sb")
            for j in range(8):
                sqp = scr.tile([P, D], f32, tag="sq")
                nc.scalar.activation(
                    sqp[:], s_sb2[:, j, :], AF.Square,
                    accum_out=pnsq[:, j:j + 1])
            nc.vector.reciprocal(ptmp[:], pnsq[:])
            nc.scalar.sqrt(pinv[:], ptmp[:])
            for j in range(8):
                nc.vector.tensor_scalar(
                    shat_sb[:, j, :], s_sb2[:, j, :], pinv[:, j:j + 1],
                    None, OP.mult)
            nc.sync.dma_start(out=shat_v, in_=shat_sb[:])
            for j in range(8):
                nc.sync.dma_start_transpose(
                    out=sT[:, :, j * P:(j + 1) * P], in_=shat_sb[:, j, :])

            # ================= Phase B =================
            with tc.tile_pool(name="psB", bufs=2, space="PSUM") as psB:
                for g in range(ng):
                    Gts = []
                    for t in range(group):
                        n = g * group + t
                        Gt = gat.tile([P, D], bf16, tag=f"G{t}",
                                      name=f"G_{n}")
                        nc.gpsimd.indirect_dma_start(
                            out=Gt[:], out_offset=None,
                            in_=shat_dram[:],
                            in_offset=bass.IndirectOffsetOnAxis(
                                ap=lab_i[:, n:n + 1], axis=0))
                        Gts.append(Gt)
                    for t in range(group):
                        n = g * group + t
                        cos_ps = psB.tile([P, C], f32, tag="cos")
                        for h in range(2):
                            for c in range(2):
                                nc.tensor.matmul(
                                    out=cos_ps[:, h * 512:(h + 1) * 512],
                                    lhsT=eT[:, n, c, :],
                                    rhs=sT[:, c, h * 512:(h + 1) * 512],
                                    start=(c == 0), stop=(c == 1))
                        exps = scr.tile([P, C], bf16, tag="exps")
                        nc.scalar.activation(
                            exps[:], cos_ps[:], AF.Exp, scale=INV_T,
                            accum_out=sumexp[:, n:n + 1])
                        mdot = scr.tile([P, D], bf16, tag="mdot")
                        nc.vector.tensor_tensor(
                            mdot[:], e_bf[:, n * D:(n + 1) * D], Gts[t][:],
                            op=OP.mult)
                        nc.vector.reduce_sum(m_all[:, n:n + 1], mdot[:],
                                             axis=mybir.AxisListType.X)

            # ================= Phase B epilogue (batched [P, nt]) ========
            b1 = big.tile([P, nt], f32, tag="b1")
            b2 = big.tile([P, nt], f32, tag="b2")
            b3 = big.tile([P, nt], f32, tag="b3")
            b4 = big.tile([P, nt], f32, tag="b4")
            mask = big.tile([P, nt], mybir.dt.uint8, tag="mask")
            phi_f = big.tile([P, nt], f32, tag="phi_f")

            nc.vector.tensor_tensor(b1[:], m_all[:], m_all[:], op=OP.mult)
            nc.vector.tensor_scalar(b1[:], b1[:], -1.0, 1.0, OP.mult, OP.add)
            nc.vector.tensor_scalar_max(b1[:], b1[:], 0.0)
            nc.scalar.sqrt(b2[:], b1[:])                        # sin
            nc.vector.tensor_scalar_mul(b3[:], m_all[:], COS_M)
            nc.vector.tensor_scalar(b2[:], b2[:], -SIN_M, None, OP.mult)
            nc.vector.tensor_add(b3[:], b3[:], b2[:])           # phi
            nc.vector.tensor_scalar(mask[:], m_all[:], TH, None, OP.is_gt)
            nc.vector.tensor_scalar(b4[:], m_all[:], -MARGIN, None, OP.add)
            nc.vector.select(phi_f[:], mask[:], b3[:], b4[:])
            nc.scalar.activation(b1[:], m_all[:], AF.Exp, scale=INV_T)
            nc.scalar.activation(b2[:], phi_f[:], AF.Exp, scale=INV_T)
            nc.vector.tensor_sub(b1[:], sumexp[:], b1[:])
            nc.vector.tensor_add(b1[:], b1[:], b2[:])           # Z
            nc.scalar.activation(b2[:], b1[:], AF.Ln, scale=1.0)
            nc.vector.tensor_scalar_mul(b3[:], phi_f[:], INV_T)
            nc.vector.tensor_sub(b2[:], b2[:], b3[:])           # nll
            part = big.tile([P, 1], f32, tag="part")
            nc.vector.reduce_sum(part[:], b2[:], axis=mybir.AxisListType.X)
            nc.sync.dma_start(out=out[:], in_=part[:])

    nc.compile()
    return nc


_NC_CACHE = {}


def kernel(embeddings, labels, num_classes=None, **_ignored):
    embeddings = np.ascontiguousarray(embeddings, dtype=np.float32)
    labels = np.ascontiguousarray(labels, dtype=np.int32)
    B = embeddings.shape[0]
    BL = B // NCORES

    if "nc" not in _NC_CACHE:
        _NC_CACHE["nc"] = build()
    nc = _NC_CACHE["nc"]

    in_maps = [{"embeddings": embeddings[i * BL:(i + 1) * BL],
                "labels": labels[i * BL:(i + 1) * BL]}
               for i in range(NCORES)]
    res = run_bass_kernel_spmd(nc, in_maps, list(range(NCORES)))
    total = 0.0
    for i in range(NCORES):
        total += res.results[i]["out"].astype(np.float64).sum()
    return np.float32(total / B)

